# revision 25
# baseline (speedup 1.0000x reference)
"""Trainium2 Bass kernel for nn_DSF_31688268710552 (soft-median pooling).

Reference computation (B,C,H,W = 4,64,64,64):
  ab = 100*tanh(conv3x3(x))        -> alpha, beta  [B,1,H,W]
  d[b,i,j,h,w] = beta*x_i - x_j    (CxC pairwise per location)
  sabs = d * tanh(alpha*d)
  tmp_j = mean_i sabs
  sam = softmax_j(clip(-tmp*alpha, +-100))
  out = sum_j x_j * sam_j          [B,1,H,W]

Identity used:  d*tanh(a*d) = silu(2*a*d)/a - d, so
  logits_j = -alpha*tmp_j = -(1/C)*sum_i silu(2*alpha*d_ij)
             + (alpha*beta/C)*S_x - alpha*x_j,   S_x = sum_i x_i
(no division by alpha; alpha==0 gives logits 0 == reference).

Sharding: 8 cores <- (batch, H-half): core k handles b=k//2, rows
[32*(k%2), 32*(k%2)+32).  2048 locations/core; conv halo via host zero-pad.

Per-core pipeline (chunk-interleaved so compute starts early):
  conv3x3 as 24 fp32 matmuls (vertical tap pairs stacked to K=128), per
    128-col chunk of 4 location-tiles.
  d-build: psum_d[n,(j,i)] = u[n,i]-x[n,j] with u=beta*x, via matmuls of the
    stationary data [u_cm; x_cm] against a constant +-1 selector (fp16 hi/lo
    split: two accumulating fp16 matmuls, error ~2^-22 -- fp32-grade).
  Phase A per half-tile: ScalarE Silu(2*alpha*d) (per-partition AP scale),
    VectorE segmented reduce over i, logits assembly (GPSIMD+DVE) + clip.
  Phase B per tile: ScalarE Exp(+accum), DVE/GPSIMD normalize + weighted sum.
  Phase B is pinned after the last Silu so ScalarE loads each act table once.
"""

import numpy as np

import concourse.bass as bass
import concourse.tile as tile
from concourse import mybir
from concourse.bass_utils import run_bass_kernel_spmd
from concourse.tile_rust import add_dep_helper

AF = mybir.ActivationFunctionType
OP = mybir.AluOpType
F32 = mybir.dt.float32
F16 = mybir.dt.float16

B, C, H, W = 4, 64, 64, 64
NCORES = 8
HLOC = 32                  # H rows per core
NLOC = HLOC * W            # 2048 locations per core
NTILES = NLOC // 128       # 16
CC = C * C                 # 4096
NCHUNK = 4                 # prologue/conv chunks (4 tiles each)
CHW = NLOC // NCHUNK       # 512 columns per chunk
ROWS = CHW // W            # 8 h rows per chunk


def _split_waits(nc, maxw=1):
    """walrus in this env rejects >maxw sync-waits per instruction.  Move
    excess waits onto single-wait NoOp carriers just before, same engine."""
    n = 0
    for f in nc.m.functions:
        for bb in f.blocks:
            newlist = []
            for inst in bb.instructions:
                si = inst.sync_info
                if si is not None and si.on_wait and len(si.on_wait) > maxw:
                    waits = list(si.on_wait)
                    extra, keep = waits[:-maxw], waits[-maxw:]
                    for k, w in enumerate(extra):
                        carrier = mybir.InstNoOp(
                            name=f"{inst.name}-wsplit{k}",
                            engine=inst.engine,
                            sync_info=mybir.SyncInfo(on_wait=[w], on_update=[]),
                            ins=[],
                            outs=[],
                        )
                        newlist.append(carrier)
                        n += 1
                    si.on_wait[:] = keep
                newlist.append(inst)
            bb.instructions[:] = newlist
    return n


def _build_bmat16():
    """Selector [128, 4096] fp16: col (j*64+i) = +1 at row i, -1 at row 64+j."""
    bm = np.zeros((128, CC), dtype=np.float16)
    cols = np.arange(CC)
    i = cols % C
    j = cols // C
    bm[i, cols] = 1.0
    bm[C + j, cols] = -1.0
    return bm


def _build_nc():
    nc = bass.Bass()

    # xc2: channel-major with halo, double-stacked for vertical tap pairs:
    #   rows 0:64 = x[c, r, w] (padded rows r=0..33), rows 64:128 = x[c, r+1, w]
    xc2_d = nc.dram_tensor("xc2", [128, HLOC + 2, W + 2], F32, kind="ExternalInput")
    xt_d = nc.dram_tensor("xt", [NLOC, C], F32, kind="ExternalInput")
    xchi_d = nc.dram_tensor("xchi", [C, NLOC], F16, kind="ExternalInput")
    xclo_d = nc.dram_tensor("xclo", [C, NLOC], F16, kind="ExternalInput")
    # conv weights for tap pairs: cols 2p+o = [w(dy=-1,dx); w(dy=0,dx)][:,o],
    # cols 6+2p+o = [w(dy=+1,dx); 0][:,o], p = dx+1
    wt2_d = nc.dram_tensor("wt2", [128, 12], F32, kind="ExternalInput")
    cb_d = nc.dram_tensor("cb", [2, 1], F32, kind="ExternalInput")
    bm_d = nc.dram_tensor("bmat", [128, CC], F16, kind="ExternalInput")
    y_d = nc.dram_tensor("y", [NLOC], F32, kind="ExternalOutput")
    bbt_d = nc.dram_tensor("bb_tmp", [NLOC], F32)  # bounce for beta bcast

    with tile.TileContext(nc) as tc:
        with (
            tc.tile_pool(name="singles", bufs=1) as singles,
            tc.tile_pool(name="swp", bufs=2) as swp,
            tc.tile_pool(name="work", bufs=3) as work,
            tc.tile_pool(name="tiny", bufs=4) as tiny,
            tc.tile_pool(name="dpsum", bufs=3, space="PSUM") as dpsum,
            tc.tile_pool(name="cpsum", bufs=2, space="PSUM") as cpsum,
        ):
            # ---- load inputs (conv deps first: conv is the startup path) ----
            xc2 = singles.tile([128, HLOC + 2, W + 2], F32)
            nc.sync.dma_start(out=xc2, in_=xc2_d[:, :, :])
            wt2 = singles.tile([128, 12], F32)
            nc.sync.dma_start(out=wt2, in_=wt2_d[:, :])
            cb = singles.tile([2, 1], F32)
            nc.sync.dma_start(out=cb, in_=cb_d[:, :])
            lhsT_hi = singles.tile([128, NLOC], F16)
            lhsT_lo = singles.tile([128, NLOC], F16)
            nc.sync.dma_start(out=lhsT_hi[C:128, :], in_=xchi_d[:, :])
            nc.sync.dma_start(out=lhsT_lo[C:128, :], in_=xclo_d[:, :])
            xt = singles.tile([128, NTILES, C], F32)
            nc.sync.dma_start(out=xt, in_=xt_d.rearrange("(t p) c -> p t c", p=128))
            bmat = singles.tile([128, CC], F16)
            nc.sync.dma_start(out=bmat, in_=bm_d[:, :])

            ab = singles.tile([2, NLOC], F32)
            at_col = singles.tile([128, NTILES], F32)
            bt_col = singles.tile([128, NTILES], F32)
            s2a = singles.tile([128, NTILES], F32)    # 2*alpha
            nega = singles.tile([128, NTILES], F32)   # -alpha
            prod = singles.tile([128, NTILES], F32)   # alpha*beta/C
            bb = singles.tile([C, NLOC], F32)
            ucm = singles.tile([C, NLOC], F32)
            uhi32 = singles.tile([C, NLOC], F32)
            xcint = xc2[0:C, 1 : HLOC + 1, 1 : W + 1]  # interior [64,32,64]

            Lc_all = singles.tile([128, NTILES, C], F32)
            mneg_all = singles.tile([128, NTILES], F32)
            sxc = singles.tile([128, NTILES], F32)
            outb = singles.tile([128, NTILES], F32)

            state = {"last_silu": None}

            def prologue(cnk):
                s = slice(cnk * CHW, (cnk + 1) * CHW)
                sc = slice(cnk * NCHUNK, (cnk + 1) * NCHUNK)  # 4 tile-columns
                r0 = cnk * ROWS

                # -- conv chunk: 6 fp32 matmuls into the conv psum bank --
                conv_ps = cpsum.tile([2, CHW], F32, tag="cv")
                for p in range(3):  # dx = p-1
                    dx = p - 1
                    nc.tensor.matmul(
                        conv_ps,
                        wt2[:, 2 * p : 2 * p + 2],
                        xc2[:, r0 : r0 + ROWS, 1 + dx : 1 + W + dx],
                        start=(p == 0),
                        stop=False,
                    )
                    nc.tensor.matmul(
                        conv_ps,
                        wt2[:, 6 + 2 * p : 8 + 2 * p],
                        xc2[:, r0 + 2 : r0 + 2 + ROWS, 1 + dx : 1 + W + dx],
                        start=False,
                        stop=(p == 2),
                    )
                nc.scalar.activation(
                    out=ab[:, s], in_=conv_ps, func=AF.Tanh, bias=cb, scale=1.0
                )

                # -- alpha/beta per-partition columns for this chunk --
                for t in range(cnk * NCHUNK, (cnk + 1) * NCHUNK):
                    nc.sync.dma_start(
                        out=at_col[:, t : t + 1], in_=ab[0:1, t * 128 : (t + 1) * 128]
                    )
                    nc.sync.dma_start(
                        out=bt_col[:, t : t + 1], in_=ab[1:2, t * 128 : (t + 1) * 128]
                    )
                nc.vector.tensor_scalar_mul(s2a[:, sc], at_col[:, sc], 200.0)
                nc.vector.tensor_scalar_mul(nega[:, sc], at_col[:, sc], -100.0)
                nc.vector.tensor_mul(prod[:, sc], at_col[:, sc], bt_col[:, sc])
                nc.vector.tensor_scalar_mul(prod[:, sc], prod[:, sc], 10000.0 / C)

                # -- u = 100*beta*x (channel-major) + fp16 hi/lo split --
                nc.sync.dma_start(out=bbt_d[s], in_=ab[1:2, s])
                v = bbt_d[s]
                src = bass.AP(tensor=v.tensor, offset=v.offset, ap=[[0, C]] + list(v.ap))
                nc.sync.dma_start(out=bb[:, s], in_=src)
                nc.vector.scalar_tensor_tensor(
                    out=ucm[:, s].rearrange("p (h w) -> p h w", w=W),
                    in0=xcint[:, r0 : r0 + ROWS, :],
                    scalar=100.0,
                    in1=bb[:, s].rearrange("p (h w) -> p h w", w=W),
                    op0=OP.mult,
                    op1=OP.mult,
                )
                nc.vector.tensor_copy(lhsT_hi[0:C, s], ucm[:, s])
                nc.scalar.copy(uhi32[:, s], lhsT_hi[0:C, s])
                nc.vector.tensor_sub(uhi32[:, s], ucm[:, s], uhi32[:, s])
                nc.vector.tensor_copy(lhsT_lo[0:C, s], uhi32[:, s])

            def tiles(cnk):
                for t in range(cnk * NCHUNK, (cnk + 1) * NCHUNK):
                    xt_t = xt[:, t, :]
                    nc.vector.reduce_sum(
                        sxc[:, t : t + 1], xt_t, axis=mybir.AxisListType.X
                    )

                    sw = swp.tile([128, CC], F32, tag="sw")
                    for q in range(4):  # quarter-tiles: j in [16q, 16q+16)
                        psum_d = dpsum.tile([128, CC // 4], F32, tag="d")
                        # hi passes for both banks first, then lo (fewer LDW)
                        for part, st in ((lhsT_hi, True), (lhsT_lo, False)):
                            for bk in range(2):
                                bs = slice(
                                    q * CC // 4 + bk * 512,
                                    q * CC // 4 + (bk + 1) * 512,
                                )
                                ps = slice(bk * 512, (bk + 1) * 512)
                                nc.tensor.matmul(
                                    psum_d[:, ps],
                                    part[:, t * 128 : (t + 1) * 128],
                                    bmat[:, bs],
                                    start=st,
                                    stop=not st,
                                )
                        state["last_silu"] = nc.scalar.activation(
                            out=sw[:, q * CC // 4 : (q + 1) * CC // 4],
                            in_=psum_d,
                            func=AF.Silu,
                            scale=s2a[:, t : t + 1],
                        )
                    A = work.tile([128, C], F32, tag="A")
                    nc.vector.reduce_sum(
                        A,
                        sw.rearrange("p (j i) -> p j i", i=C),
                        axis=mybir.AxisListType.X,
                    )

                    # logits: L = -(1/C)*A + (alpha*beta/C)*S_x - alpha*x_j
                    c1 = tiny.tile([128, 1], F32, tag="c1")
                    nc.gpsimd.tensor_mul(c1, prod[:, t : t + 1], sxc[:, t : t + 1])
                    T = work.tile([128, C], F32, tag="T")
                    nc.vector.tensor_scalar(
                        T, xt_t, nega[:, t : t + 1], c1, op0=OP.mult, op1=OP.add
                    )
                    As = work.tile([128, C], F32, tag="As")
                    nc.gpsimd.tensor_scalar_mul(As, A, -1.0 / C)
                    L = work.tile([128, C], F32, tag="L")
                    nc.gpsimd.tensor_add(L, As, T)
                    nc.gpsimd.tensor_scalar(
                        Lc_all[:, t, :], L, 100.0, -100.0, op0=OP.min, op1=OP.max
                    )
                    nc.vector.tensor_reduce(
                        mneg_all[:, t : t + 1],
                        Lc_all[:, t, :],
                        axis=mybir.AxisListType.X,
                        op=OP.max,
                        negate=True,
                    )

            # one-chunk software lookahead: prologue(c+1) is emitted before
            # tiles(c) so its conv/DMA chain hides behind chunk c's compute
            prologue(0)
            for cnk in range(NCHUNK):
                if cnk + 1 < NCHUNK:
                    prologue(cnk + 1)
                tiles(cnk)
            last_silu = state["last_silu"]

            # ---- Phase B: softmax + weighted sum (Exp table loaded once) ----
            for t in range(NTILES):
                xt_t = xt[:, t, :]
                e = work.tile([128, C], F32, tag="e")
                Z = tiny.tile([128, 1], F32, tag="Z")
                exp_inst = nc.scalar.activation(
                    out=e,
                    in_=Lc_all[:, t, :],
                    func=AF.Exp,
                    bias=mneg_all[:, t : t + 1],
                    scale=1.0,
                    accum_out=Z,
                )
                if last_silu is not None:
                    add_dep_helper(
                        exp_inst.ins,
                        last_silu.ins,
                        reason="exp after all silus (act table)",
                    )
                rz = tiny.tile([128, 1], F32, tag="rz")
                nc.vector.reciprocal(rz, Z)
                xe = work.tile([128, C], F32, tag="xe")
                O = tiny.tile([128, 1], F32, tag="O")
                nc.vector.scalar_tensor_tensor(
                    out=xe, in0=xt_t, scalar=1.0, in1=e, op0=OP.mult, op1=OP.mult,
                    accum_out=O,
                )
                nc.gpsimd.tensor_mul(outb[:, t : t + 1], O, rz)

            nc.sync.dma_start(out=y_d.rearrange("(t p) -> p t", p=128), in_=outb)

    _split_waits(nc)
    return nc


_NC_CACHE = {}


def _get_nc():
    if "nc" not in _NC_CACHE:
        _NC_CACHE["nc"] = _build_nc()
    return _NC_CACHE["nc"]


def _prep_in_maps(x, conv_w, conv_b):
    x = np.ascontiguousarray(np.asarray(x, dtype=np.float32))
    conv_w = np.asarray(conv_w, dtype=np.float32)
    conv_b = np.asarray(conv_b, dtype=np.float32)

    # H padded by 2 at the bottom: the +1-shifted tap-pair rows reach h=H+1
    xpad = np.pad(x, ((0, 0), (0, 0), (1, 2), (1, 1)))  # [B, C, 67, 66]

    wt2 = np.zeros((128, 12), dtype=np.float32)
    for p in range(3):  # dx = p-1, kw = p
        wt2[0:C, 2 * p : 2 * p + 2] = conv_w[:, :, 0, p].T       # dy=-1 (kh=0)
        wt2[C:128, 2 * p : 2 * p + 2] = conv_w[:, :, 1, p].T     # dy= 0 (kh=1)
        wt2[0:C, 6 + 2 * p : 8 + 2 * p] = conv_w[:, :, 2, p].T   # dy=+1 (kh=2)
    cb = conv_b.reshape(2, 1).astype(np.float32)
    bmat = _build_bmat16()

    in_maps = []
    for k in range(NCORES):
        b, half = k // 2, k % 2
        h0 = HLOC * half
        xc2 = np.empty((128, HLOC + 2, W + 2), dtype=np.float32)
        xc2[0:C] = xpad[b, :, h0 : h0 + HLOC + 2, :]
        xc2[C:128] = xpad[b, :, h0 + 1 : h0 + HLOC + 3, :]
        xt = np.ascontiguousarray(
            x[b, :, h0 : h0 + HLOC, :].transpose(1, 2, 0).reshape(NLOC, C)
        )
        xcint = np.ascontiguousarray(x[b, :, h0 : h0 + HLOC, :].reshape(C, NLOC))
        xchi = xcint.astype(np.float16)
        xclo = (xcint - xchi.astype(np.float32)).astype(np.float16)
        in_maps.append(
            {
                "xc2": xc2,
                "xt": xt,
                "xchi": xchi,
                "xclo": xclo,
                "wt2": wt2,
                "cb": cb,
                "bmat": bmat,
            }
        )
    return in_maps


def kernel(x, conv_w, conv_b):
    in_maps = _prep_in_maps(x, conv_w, conv_b)
    nc = _get_nc()
    res = run_bass_kernel_spmd(nc, in_maps, list(range(NCORES)))

    out = np.empty((B, 1, H, W), dtype=np.float32)
    for k in range(NCORES):
        b, half = k // 2, k % 2
        h0 = HLOC * half
        out[b, 0, h0 : h0 + HLOC, :] = res.results[k]["y"].reshape(HLOC, W)
    return out


# revision 45
# speedup vs baseline: 6115.9831x; 6115.9831x over previous
"""Trainium2 Bass kernel for nn_DSF_31688268710552 (soft-median pooling).

Reference computation (B,C,H,W = 4,64,64,64):
  ab = 100*tanh(conv3x3(x))        -> alpha, beta  [B,1,H,W]
  d[b,i,j,h,w] = beta*x_i - x_j    (CxC pairwise per location)
  sabs = d * tanh(alpha*d)
  tmp_j = mean_i sabs
  sam = softmax_j(clip(-tmp*alpha, +-100))
  out = sum_j x_j * sam_j          [B,1,H,W]

Identity used:  d*tanh(a*d) = silu(2*a*d)/a - d, so
  logits_j = -alpha*tmp_j = -(1/C)*sum_i silu(2*alpha*d_ij)
             + (alpha*beta/C)*S_x - alpha*x_j,   S_x = sum_i x_i
(no division by alpha; alpha==0 gives logits 0 == reference).

Sharding: 8 cores <- (batch, H-half): core k handles b=k//2, rows
[32*(k%2), 32*(k%2)+32).  2048 locations/core; conv halo via host zero-pad.

Per-core pipeline (chunk-interleaved so compute starts early):
  conv3x3 as 24 fp32 matmuls (vertical tap pairs stacked to K=128), per
    128-col chunk of 4 location-tiles.
  d-build: psum_d[n,(j,i)] = u[n,i]-x[n,j] with u=beta*x, via matmuls of the
    stationary data [u_cm; x_cm] against a constant +-1 selector (fp16 hi/lo
    split: two accumulating fp16 matmuls, error ~2^-22 -- fp32-grade).
  Phase A per half-tile: ScalarE Silu(2*alpha*d) (per-partition AP scale),
    VectorE segmented reduce over i, logits assembly (GPSIMD+DVE) + clip.
  Phase B per tile: ScalarE Exp(+accum), DVE/GPSIMD normalize + weighted sum.
  Phase B is pinned after the last Silu so ScalarE loads each act table once.
"""

import numpy as np

import concourse.bass as bass
import concourse.tile as tile
from concourse import mybir
from concourse.bass_utils import run_bass_kernel_spmd
from concourse.tile_rust import add_dep_helper

AF = mybir.ActivationFunctionType
OP = mybir.AluOpType
F32 = mybir.dt.float32
F16 = mybir.dt.float16

B, C, H, W = 4, 64, 64, 64
NCORES = 8
HLOC = 32                  # H rows per core
NLOC = HLOC * W            # 2048 locations per core
NTILES = NLOC // 128       # 16
CC = C * C                 # 4096
NCHUNK = 4                 # prologue/conv chunks (4 tiles each)
CHW = NLOC // NCHUNK       # 512 columns per chunk
ROWS = CHW // W            # 8 h rows per chunk


def _split_waits(nc, maxw=1):
    """walrus in this env rejects >maxw sync-waits per instruction.  Move
    excess waits onto single-wait NoOp carriers just before, same engine."""
    n = 0
    for f in nc.m.functions:
        for bb in f.blocks:
            newlist = []
            for inst in bb.instructions:
                si = inst.sync_info
                if si is not None and si.on_wait and len(si.on_wait) > maxw:
                    waits = list(si.on_wait)
                    extra, keep = waits[:-maxw], waits[-maxw:]
                    for k, w in enumerate(extra):
                        carrier = mybir.InstNoOp(
                            name=f"{inst.name}-wsplit{k}",
                            engine=inst.engine,
                            sync_info=mybir.SyncInfo(on_wait=[w], on_update=[]),
                            ins=[],
                            outs=[],
                        )
                        newlist.append(carrier)
                        n += 1
                    si.on_wait[:] = keep
                newlist.append(inst)
            bb.instructions[:] = newlist
    return n


def _build_bmat16():
    """Selector [128, 4096] fp16: col (j*64+i) = +1 at row i, -1 at row 64+j."""
    bm = np.zeros((128, CC), dtype=np.float16)
    cols = np.arange(CC)
    i = cols % C
    j = cols // C
    bm[i, cols] = 1.0
    bm[C + j, cols] = -1.0
    return bm


def _build_nc(repeat=1):
    nc = bass.Bass()

    # xc2: channel-major with halo, double-stacked for vertical tap pairs:
    #   rows 0:64 = x[c, r, w] (padded rows r=0..33), rows 64:128 = x[c, r+1, w]
    xc2_d = nc.dram_tensor("xc2", [128, HLOC + 2, W + 2], F32, kind="ExternalInput")
    xt_d = nc.dram_tensor("xt", [NLOC, C], F32, kind="ExternalInput")
    xchi_d = nc.dram_tensor("xchi", [C, NLOC], F16, kind="ExternalInput")
    xclo_d = nc.dram_tensor("xclo", [C, NLOC], F16, kind="ExternalInput")
    # conv weights for tap pairs: cols 2p+o = [w(dy=-1,dx); w(dy=0,dx)][:,o],
    # cols 6+2p+o = [w(dy=+1,dx); 0][:,o], p = dx+1; fp16 hi/lo split
    wt2h_d = nc.dram_tensor("wt2h", [128, 12], F16, kind="ExternalInput")
    wt2l_d = nc.dram_tensor("wt2l", [128, 12], F16, kind="ExternalInput")
    xc2h_d = nc.dram_tensor("xc2h", [128, HLOC + 2, W + 2], F16, kind="ExternalInput")
    xc2l_d = nc.dram_tensor("xc2l", [128, HLOC + 2, W + 2], F16, kind="ExternalInput")
    cb_d = nc.dram_tensor("cb", [2, 1], F32, kind="ExternalInput")
    bm_d = nc.dram_tensor("bmat", [128, CC], F16, kind="ExternalInput")
    y_d = nc.dram_tensor("y", [NLOC], F32, kind="ExternalOutput")
    bbt_d = nc.dram_tensor("bb_tmp", [NLOC], F32)  # bounce for beta bcast

    with tile.TileContext(nc) as tc:
        with (
            tc.tile_pool(name="singles", bufs=1) as singles,
            tc.tile_pool(name="swp", bufs=3) as swp,
            tc.tile_pool(name="work", bufs=3) as work,
            tc.tile_pool(name="tiny", bufs=4) as tiny,
            tc.tile_pool(name="dpsum", bufs=3, space="PSUM") as dpsum,
            tc.tile_pool(name="cpsum", bufs=2, space="PSUM") as cpsum,
        ):
            # ---- load inputs (conv deps first: conv is the startup path) ----
            wt2h = singles.tile([128, 12], F16)
            nc.sync.dma_start(out=wt2h, in_=wt2h_d[:, :])
            wt2l = singles.tile([128, 12], F16)
            nc.sync.dma_start(out=wt2l, in_=wt2l_d[:, :])
            cb = singles.tile([2, 1], F32)
            nc.sync.dma_start(out=cb, in_=cb_d[:, :])
            xc2h = singles.tile([128, HLOC + 2, W + 2], F16)
            xc2l = singles.tile([128, HLOC + 2, W + 2], F16)
            # chunk-0 conv needs only the first ROWS+2 halo rows; load them
            # first so the conv (the startup critical path) starts immediately
            nc.sync.dma_start(
                out=xc2h[:, 0 : ROWS + 4, :], in_=xc2h_d[:, 0 : ROWS + 4, :]
            )
            nc.sync.dma_start(
                out=xc2l[:, 0 : ROWS + 4, :], in_=xc2l_d[:, 0 : ROWS + 4, :]
            )
            # bulk loads go on the SWDGE queue so the SP queue stays free for
            # the latency-critical alpha/beta scatter chain after conv 0
            nc.gpsimd.dma_start(
                out=xc2h[:, ROWS + 4 :, :], in_=xc2h_d[:, ROWS + 4 :, :]
            )
            nc.gpsimd.dma_start(
                out=xc2l[:, ROWS + 4 :, :], in_=xc2l_d[:, ROWS + 4 :, :]
            )
            xc2 = singles.tile([128, HLOC + 2, W + 2], F32)
            nc.gpsimd.dma_start(out=xc2, in_=xc2_d[:, :, :])
            lhsT_hi = singles.tile([128, NLOC], F16)
            lhsT_lo = singles.tile([128, NLOC], F16)
            nc.gpsimd.dma_start(out=lhsT_hi[C:128, :], in_=xchi_d[:, :])
            nc.gpsimd.dma_start(out=lhsT_lo[C:128, :], in_=xclo_d[:, :])
            xt = singles.tile([128, NTILES, C], F32)
            nc.gpsimd.dma_start(out=xt, in_=xt_d.rearrange("(t p) c -> p t c", p=128))
            bmat = singles.tile([128, CC], F16)
            nc.gpsimd.dma_start(out=bmat, in_=bm_d[:, :])

            ab = singles.tile([2, NLOC], F32)
            at_col = singles.tile([128, NTILES], F32)
            bt_col = singles.tile([128, NTILES], F32)
            s2a = singles.tile([128, NTILES], F32)    # 2*alpha
            nega = singles.tile([128, NTILES], F32)   # -alpha
            prod = singles.tile([128, NTILES], F32)   # alpha*beta/C
            bb = singles.tile([C, NLOC], F32)
            ucm = singles.tile([C, NLOC], F32)
            uhi32 = singles.tile([C, NLOC], F32)
            xcint = xc2[0:C, 1 : HLOC + 1, 1 : W + 1]  # interior [64,32,64]

            Lc_all = singles.tile([128, NTILES, C], F32)
            mneg_all = singles.tile([128, NTILES], F32)
            sxc = singles.tile([128, NTILES], F32)
            outb = singles.tile([128, NTILES], F32)

            # PE warmup: tiny matmuls on a zeroed tile start the HAM p-state
            # ramp clock (~3.4us to full clock) before the conv arrives
            wm = singles.tile([128, C], F32)
            nc.vector.memset(wm, 0.0)
            wm_ps = cpsum.tile([2, CHW], F32, tag="cv")
            for _ in range(10):
                nc.tensor.matmul(
                    wm_ps[0:2, 0:C], wm[:, 0:2], wm, start=True, stop=True
                )

            state = {"last_silu": None}

            def prologue(cnk):
                s = slice(cnk * CHW, (cnk + 1) * CHW)
                sc = slice(cnk * NCHUNK, (cnk + 1) * NCHUNK)  # 4 tile-columns
                r0 = cnk * ROWS

                # -- conv chunk: 18 fp16 matmuls (xh*wh + xh*wl + xl*wh)
                conv_ps = cpsum.tile([2, CHW], F32, tag="cv")
                first = True
                for p in range(3):  # dx = p-1
                    dx = p - 1
                    for xsrc, wsrc in (
                        (xc2h, wt2h), (xc2h, wt2l), (xc2l, wt2h)
                    ):
                        nc.tensor.matmul(
                            conv_ps,
                            wsrc[:, 2 * p : 2 * p + 2],
                            xsrc[:, r0 : r0 + ROWS, 1 + dx : 1 + W + dx],
                            start=first,
                            stop=False,
                        )
                        first = False
                        nc.tensor.matmul(
                            conv_ps,
                            wsrc[:, 6 + 2 * p : 8 + 2 * p],
                            xsrc[:, r0 + 2 : r0 + 2 + ROWS, 1 + dx : 1 + W + dx],
                            start=False,
                            stop=(p == 2 and wsrc is wt2h and xsrc is xc2l),
                        )
                nc.scalar.activation(
                    out=ab[:, s], in_=conv_ps, func=AF.Tanh, bias=cb, scale=1.0
                )

                # -- alpha/beta per-partition columns for this chunk --
                for t in range(cnk * NCHUNK, (cnk + 1) * NCHUNK):
                    nc.sync.dma_start(
                        out=at_col[:, t : t + 1], in_=ab[0:1, t * 128 : (t + 1) * 128]
                    )
                    nc.sync.dma_start(
                        out=bt_col[:, t : t + 1], in_=ab[1:2, t * 128 : (t + 1) * 128]
                    )
                # immediate-scalar ops on (idle) GPSIMD, per tile so tile 4c's
                # silu is unblocked right after its own two scatter DMAs
                for t in range(cnk * NCHUNK, (cnk + 1) * NCHUNK):
                    st = slice(t, t + 1)
                    nc.gpsimd.tensor_scalar_mul(s2a[:, st], at_col[:, st], 200.0)
                    nc.gpsimd.tensor_scalar_mul(nega[:, st], at_col[:, st], -100.0)
                    nc.gpsimd.tensor_mul(prod[:, st], at_col[:, st], bt_col[:, st])
                    nc.gpsimd.tensor_scalar_mul(prod[:, st], prod[:, st], 10000.0 / C)

                # -- u = 100*beta*x (channel-major) + fp16 hi/lo split --
                nc.sync.dma_start(out=bbt_d[s], in_=ab[1:2, s])
                v = bbt_d[s]
                src = bass.AP(tensor=v.tensor, offset=v.offset, ap=[[0, C]] + list(v.ap))
                nc.sync.dma_start(out=bb[:, s], in_=src)
                nc.vector.scalar_tensor_tensor(
                    out=ucm[:, s].rearrange("p (h w) -> p h w", w=W),
                    in0=xcint[:, r0 : r0 + ROWS, :],
                    scalar=100.0,
                    in1=bb[:, s].rearrange("p (h w) -> p h w", w=W),
                    op0=OP.mult,
                    op1=OP.mult,
                )
                nc.vector.tensor_copy(lhsT_hi[0:C, s], ucm[:, s])
                nc.scalar.copy(uhi32[:, s], lhsT_hi[0:C, s])
                nc.gpsimd.tensor_sub(uhi32[:, s], ucm[:, s], uhi32[:, s])
                nc.vector.tensor_copy(lhsT_lo[0:C, s], uhi32[:, s])

            def tiles(t_begin, t_end):
                for t in range(t_begin, t_end):
                    xt_t = xt[:, t, :]
                    nc.vector.reduce_sum(
                        sxc[:, t : t + 1], xt_t, axis=mybir.AxisListType.X
                    )

                    # first/last tiles: per-piece segreduce so the ramp/tail
                    # critical chain is piece-pipelined, not one 4.3us reduce
                    piecewise = t in (0, NTILES - 1)
                    sw = swp.tile([128, CC], F32, tag="sw")
                    A = work.tile([128, C], F32, tag="A")
                    for qi, (c0, csz) in enumerate(
                        ((0, 1024), (1024, 1024), (2048, 1024), (3072, 1024))
                    ):
                        psum_d = dpsum.tile([128, 1024], F32, tag="d")
                        # hi passes for all banks first, then lo (fewer LDW)
                        for part, st in ((lhsT_hi, True), (lhsT_lo, False)):
                            for bk in range(csz // 512):
                                bs = slice(
                                    c0 + bk * 512, c0 + (bk + 1) * 512
                                )
                                ps = slice(bk * 512, (bk + 1) * 512)
                                nc.tensor.matmul(
                                    psum_d[:, ps],
                                    part[:, t * 128 : (t + 1) * 128],
                                    bmat[:, bs],
                                    start=st,
                                    stop=not st,
                                )
                        state["last_silu"] = nc.scalar.activation(
                            out=sw[:, c0 : c0 + csz],
                            in_=psum_d[:, 0:csz],
                            func=AF.Silu,
                            scale=s2a[:, t : t + 1],
                        )
                        if piecewise:
                            nc.vector.reduce_sum(
                                A[:, qi * 16 : (qi + 1) * 16],
                                sw[:, c0 : c0 + csz].rearrange(
                                    "p (j i) -> p j i", i=C
                                ),
                                axis=mybir.AxisListType.X,
                            )
                    if not piecewise:
                        nc.vector.reduce_sum(
                            A,
                            sw.rearrange("p (j i) -> p j i", i=C),
                            axis=mybir.AxisListType.X,
                        )

                    # logits: L = -(1/C)*A + (alpha*beta/C)*S_x - alpha*x_j
                    c1 = tiny.tile([128, 1], F32, tag="c1")
                    nc.gpsimd.tensor_mul(c1, prod[:, t : t + 1], sxc[:, t : t + 1])
                    T = work.tile([128, C], F32, tag="T")
                    nc.vector.tensor_scalar(
                        T, xt_t, nega[:, t : t + 1], c1, op0=OP.mult, op1=OP.add
                    )
                    As = work.tile([128, C], F32, tag="As")
                    nc.gpsimd.tensor_scalar_mul(As, A, -1.0 / C)
                    L = work.tile([128, C], F32, tag="L")
                    nc.gpsimd.tensor_add(L, As, T)
                    nc.gpsimd.tensor_scalar(
                        Lc_all[:, t, :], L, 100.0, -100.0, op0=OP.min, op1=OP.max
                    )
                    nc.vector.tensor_reduce(
                        mneg_all[:, t : t + 1],
                        Lc_all[:, t, :],
                        axis=mybir.AxisListType.X,
                        op=OP.max,
                        negate=True,
                    )

            # software pipeline: prologue(c) is emitted a few tiles before its
            # chunk so the conv/DMA chain hides behind earlier tiles' compute,
            # while tile 0's matmuls start right after conv chunk 0
            def tile_block(t):
                tiles(t, t + 1)

            for rep in range(repeat):  # repeat>1: A/B timing builds only
                # tile 0 jumps ahead of conv chunk 1 so the ACT/DVE pipeline
                # primes as early as possible; later prologues keep one-chunk
                # lookahead so their conv/DMA chains hide behind tile compute
                prologue(0)
                tiles(0, 1)
                prologue(1)
                tiles(1, 4)
                prologue(2)
                tiles(4, 8)
                prologue(3)
                tiles(8, 16)
            last_silu = state["last_silu"]

            # ---- Phase B: softmax + weighted sum (Exp table loaded once) ----
            for t in range(NTILES):
                xt_t = xt[:, t, :]
                e = work.tile([128, C], F32, tag="e")
                Z = tiny.tile([128, 1], F32, tag="Z")
                exp_inst = nc.scalar.activation(
                    out=e,
                    in_=Lc_all[:, t, :],
                    func=AF.Exp,
                    bias=mneg_all[:, t : t + 1],
                    scale=1.0,
                    accum_out=Z,
                )
                if last_silu is not None:
                    add_dep_helper(
                        exp_inst.ins,
                        last_silu.ins,
                        reason="exp after all silus (act table)",
                    )
                rz = tiny.tile([128, 1], F32, tag="rz")
                nc.vector.reciprocal(rz, Z)
                xe = work.tile([128, C], F32, tag="xe")
                O = tiny.tile([128, 1], F32, tag="O")
                nc.vector.scalar_tensor_tensor(
                    out=xe, in0=xt_t, scalar=1.0, in1=e, op0=OP.mult, op1=OP.mult,
                    accum_out=O,
                )
                nc.gpsimd.tensor_mul(outb[:, t : t + 1], O, rz)

            nc.sync.dma_start(out=y_d.rearrange("(t p) -> p t", p=128), in_=outb)

    _split_waits(nc)
    return nc


_NC_CACHE = {}


def _get_nc():
    if "nc" not in _NC_CACHE:
        _NC_CACHE["nc"] = _build_nc()
    return _NC_CACHE["nc"]


def _prep_in_maps(x, conv_w, conv_b):
    x = np.ascontiguousarray(np.asarray(x, dtype=np.float32))
    conv_w = np.asarray(conv_w, dtype=np.float32)
    conv_b = np.asarray(conv_b, dtype=np.float32)

    # H padded by 2 at the bottom: the +1-shifted tap-pair rows reach h=H+1
    xpad = np.pad(x, ((0, 0), (0, 0), (1, 2), (1, 1)))  # [B, C, 67, 66]

    wt2 = np.zeros((128, 12), dtype=np.float32)
    for p in range(3):  # dx = p-1, kw = p
        wt2[0:C, 2 * p : 2 * p + 2] = conv_w[:, :, 0, p].T       # dy=-1 (kh=0)
        wt2[C:128, 2 * p : 2 * p + 2] = conv_w[:, :, 1, p].T     # dy= 0 (kh=1)
        wt2[0:C, 6 + 2 * p : 8 + 2 * p] = conv_w[:, :, 2, p].T   # dy=+1 (kh=2)
    wt2h = wt2.astype(np.float16)
    wt2l = (wt2 - wt2h.astype(np.float32)).astype(np.float16)
    cb = conv_b.reshape(2, 1).astype(np.float32)
    bmat = _build_bmat16()

    in_maps = []
    for k in range(NCORES):
        b, half = k // 2, k % 2
        h0 = HLOC * half
        xc2 = np.empty((128, HLOC + 2, W + 2), dtype=np.float32)
        xc2[0:C] = xpad[b, :, h0 : h0 + HLOC + 2, :]
        xc2[C:128] = xpad[b, :, h0 + 1 : h0 + HLOC + 3, :]
        xc2h = xc2.astype(np.float16)
        xc2l = (xc2 - xc2h.astype(np.float32)).astype(np.float16)
        xt = np.ascontiguousarray(
            x[b, :, h0 : h0 + HLOC, :].transpose(1, 2, 0).reshape(NLOC, C)
        )
        xcint = np.ascontiguousarray(x[b, :, h0 : h0 + HLOC, :].reshape(C, NLOC))
        xchi = xcint.astype(np.float16)
        xclo = (xcint - xchi.astype(np.float32)).astype(np.float16)
        in_maps.append(
            {
                "xc2": xc2,
                "xc2h": xc2h,
                "xc2l": xc2l,
                "xt": xt,
                "xchi": xchi,
                "xclo": xclo,
                "wt2h": wt2h,
                "wt2l": wt2l,
                "cb": cb,
                "bmat": bmat,
            }
        )
    return in_maps


def kernel(x, conv_w, conv_b):
    in_maps = _prep_in_maps(x, conv_w, conv_b)
    nc = _get_nc()
    res = run_bass_kernel_spmd(nc, in_maps, list(range(NCORES)))

    out = np.empty((B, 1, H, W), dtype=np.float32)
    for k in range(NCORES):
        b, half = k // 2, k % 2
        h0 = HLOC * half
        out[b, 0, h0 : h0 + HLOC, :] = res.results[k]["y"].reshape(HLOC, W)
    return out
